# revision 20
# baseline (speedup 1.0000x reference)
"""Trainium2 Bass kernel for a causal attention head (softmax over the query
axis, scores scaled by sqrt(d_model)) with QKV/out projections.

Strategy: pure data-parallel over batch across 8 NeuronCores (32 batch rows
per core). All matmuls run in bf16 on the TensorEngine. Host-side
pre-processing (free, outside HW timing) transposes x and the weights into
feature-major layouts so that the kernel needs no on-chip transposes:

  per core:  xT [d, T]     (T = 32*64 = 2048 local tokens)
    GEMM1:   qkT = W_qk^T.T @ xT         -> [2d, T]  feature-major (per head)
    GEMM2:   vTok = xT.T @ W_v^T         -> [T, d]   token-major
    attn:    scoresT[k,q] = K^T.T @ Q^T  -> softmax over q = free axis
             outT[dv,q] = V.T @ A^T      -> attnOutT [d, T] feature-major
    GEMM3:   y = attnOutT.T @ W_out^T    -> [T, d]   token-major, DMA'd out

Everything is hardcoded for x:[256,64,2048], 16 heads, head_dim 128.
"""

import math
import os
from contextlib import ExitStack

import numpy as np
import ml_dtypes

import concourse.bass as bass
import concourse.bacc as bacc
import concourse.tile as tile
from concourse import mybir
from concourse.bass_utils import run_bass_kernel_spmd

BF16 = mybir.dt.bfloat16
F32 = mybir.dt.float32
bf16 = ml_dtypes.bfloat16

P = 128          # partitions / head_dim
D = 2048         # d_model
H = 16           # heads
S = 64           # sequence length
NCORES = 8
B = 256
BL = B // NCORES          # 32 batch rows per core
T = BL * S                # 2048 tokens per core
C = 512                   # chunk tokens
NCH = T // C              # 4 chunks per core
G = C // S                # 8 batch rows per chunk
NKB = D // P              # 16 contraction blocks
NF = 512                  # matmul moving free dim
NN = D // NF              # 4 n-blocks
SCALE = 1.0 / math.sqrt(D)

_CACHE: dict = {}
LAST_RESULTS = None


def _emit(ctx: ExitStack, tc: tile.TileContext, aps: dict, variant: str = "full"):
    nc = tc.nc
    xT, wqk, wv, wo, bqk, bv, bo, mask, out = (
        aps["xT"], aps["wqk"], aps["wv"], aps["wo"], aps["bqk"],
        aps["bv"], aps["bo"], aps["mask"], aps["out"],
    )

    px = ctx.enter_context(tc.tile_pool(name="px", bufs=2))
    pwqk = ctx.enter_context(tc.tile_pool(name="pwqk", bufs=3))
    pqk = ctx.enter_context(tc.tile_pool(name="pqk", bufs=6))
    pv_pool = ctx.enter_context(tc.tile_pool(name="pv", bufs=1))
    pw = ctx.enter_context(tc.tile_pool(name="pw", bufs=2))
    pao = ctx.enter_context(tc.tile_pool(name="pao", bufs=2))
    psm = ctx.enter_context(tc.tile_pool(name="psm", bufs=3))
    py = ctx.enter_context(tc.tile_pool(name="py", bufs=4))
    pc = ctx.enter_context(tc.tile_pool(name="pc", bufs=1))
    pp1 = ctx.enter_context(tc.tile_pool(name="pp1", bufs=2, space="PSUM"))
    pp2 = ctx.enter_context(tc.tile_pool(name="pp2", bufs=2, space="PSUM"))
    pps = ctx.enter_context(tc.tile_pool(name="pps", bufs=2, space="PSUM"))
    ppo = ctx.enter_context(tc.tile_pool(name="ppo", bufs=2, space="PSUM"))

    # ---- constants ----
    maskt = pc.tile([P, S], F32)
    nc.sync.dma_start(out=maskt, in_=mask)
    bqkt = pc.tile([P, 2 * H], F32)
    nc.sync.dma_start(out=bqkt, in_=bqk)
    bvt = pc.tile([P, D], BF16)
    nc.gpsimd.dma_start(
        out=bvt,
        in_=bass.AP(tensor=bv.tensor, offset=bv.offset, ap=[[0, P], bv.ap[0]]),
    )
    bot = pc.tile([P, D], F32)
    nc.gpsimd.dma_start(
        out=bot,
        in_=bass.AP(tensor=bo.tensor, offset=bo.offset, ap=[[0, P], bo.ap[0]]),
    )

    def attention_base0(c, h, qt, kt, vc, aoc):
        # all-base-0 variant: scores [64, G, S]; V in vc64 [64, G, D]
        sp = pps.tile([S, G, S], F32, tag="sp")
        for g in range(G):
            nc.tensor.matmul(
                sp[:, g, :], kt[:, g * S:(g + 1) * S], qt[:, g * S:(g + 1) * S],
                start=True, stop=True,
            )
        e = psm.tile([S, G, S], F32, tag="e")
        nc.scalar.activation(e, sp, mybir.ActivationFunctionType.Exp, scale=SCALE)
        ez = psm.tile([S, G, S], F32, tag="ez")
        mask_b = bass.AP(
            tensor=maskt.tensor, offset=maskt.offset,
            ap=[[maskt.ap[0][0], S], [0, G], maskt.ap[1]],
        )
        nc.vector.tensor_mul(ez, e, mask_b)
        z = psm.tile([S, G], F32, tag="z")
        nc.vector.reduce_sum(z, ez, axis=mybir.AxisListType.X)
        r = psm.tile([S, G], F32, tag="r")
        nc.vector.reciprocal(r, z)
        a = psm.tile([S, G, S], BF16, tag="a")
        r_b = bass.AP(tensor=r.tensor, offset=r.offset,
                      ap=[r.ap[0], r.ap[1], [0, S]])
        nc.vector.tensor_mul(a, ez, r_b)
        op = ppo.tile([P, G, S], F32, tag="op")
        for g in range(G):
            nc.tensor.matmul(
                op[:, g, :],
                vc[:, g, h * P:(h + 1) * P],
                a[:, g, :],
                start=True, stop=True,
            )
        nc.vector.tensor_copy(aoc[:, h, :], op)

    def attention(c, h, qt, kt, vc, aoc):
        if variant == "noattn":
            nc.vector.tensor_copy(aoc[:, h, :], qt)
            return
        if variant in ("base0", "full"):
            attention_base0(c, h, qt, kt, vc, aoc)
            return
        # scores^T for 8 batch rows: partitions = (half, k_tok), free = (j, q_tok)
        sp = pps.tile([P, G // 2, S], F32, tag="sp")
        for g in range(G):
            half, j = g % 2, g // 2
            nc.tensor.matmul(
                sp[half * S:(half + 1) * S, j, :],
                kt[:, g * S:(g + 1) * S],
                qt[:, g * S:(g + 1) * S],
                start=True, stop=True,
            )
        e = psm.tile([P, G // 2, S], F32, tag="e")
        nc.scalar.activation(e, sp, mybir.ActivationFunctionType.Exp, scale=SCALE)
        ez = psm.tile([P, G // 2, S], F32, tag="ez")
        mask_b = bass.AP(
            tensor=maskt.tensor, offset=maskt.offset,
            ap=[maskt.ap[0], [0, G // 2], maskt.ap[1]],
        )
        nc.vector.tensor_mul(ez, e, mask_b)
        z = psm.tile([P, G // 2], F32, tag="z")
        nc.vector.reduce_sum(z, ez, axis=mybir.AxisListType.X)
        r = psm.tile([P, G // 2], F32, tag="r")
        nc.vector.reciprocal(r, z)
        a = psm.tile([P, G // 2, S], BF16, tag="a")
        r_b = bass.AP(tensor=r.tensor, offset=r.offset,
                      ap=[r.ap[0], r.ap[1], [0, S]])
        nc.vector.tensor_mul(a, ez, r_b)

        if variant == "nopv":
            nc.vector.tensor_copy(aoc[:, h, 0:G // 2 * S], a)
            nc.vector.tensor_copy(aoc[:, h, G // 2 * S:], a)
            return

        op = ppo.tile([P, G, S], F32, tag="op")
        for g in range(G):
            half, j = g % 2, g // 2
            nc.tensor.matmul(
                op[:, g, :],
                vc[half * S:(half + 1) * S, g // 2, h * P:(h + 1) * P],
                a[half * S:(half + 1) * S, j, :],
                start=True, stop=True,
            )
        nc.vector.tensor_copy(aoc[:, h, :], op)

    for c in range(NCH):
        # ---- load x chunk (feature-major) ----
        xc = px.tile([P, NKB, C], BF16, tag="xc")
        for kb in range(NKB):
            nc.sync.dma_start(
                out=xc[:, kb, :],
                in_=xT[kb * P:(kb + 1) * P, c * C:(c + 1) * C],
            )

        # ---- GEMM2: vTok chunk, token-major ----
        if variant in ("base0", "full"):
            # vc64 [64 partitions = s, G batch rows, D]: every row at base 0
            vc = pv_pool.tile([S, G, D], BF16, tag="vc")
            for n in range(NN):
                wvn = pw.tile([P, NKB, NF], BF16, tag="w")
                nc.sync.dma_start(
                    out=wvn,
                    in_=wv[:, n * NF:(n + 1) * NF].rearrange("(kb p) n -> p kb n", p=P),
                )
                for g in range(G):
                    pg = pp2.tile([P, NF], F32, tag="pg")
                    for kb in range(NKB):
                        nc.tensor.matmul(
                            pg[0:S], xc[:, kb, g * S:(g + 1) * S], wvn[:, kb, :],
                            start=(kb == 0), stop=(kb == NKB - 1),
                        )
                    nc.vector.tensor_add(
                        vc[:, g, n * NF:(n + 1) * NF], pg[0:S],
                        bvt[0:S, n * NF:(n + 1) * NF],
                    )
        else:
            vc = pv_pool.tile([P, C // P, D], BF16, tag="vc")
            for n in range(NN):
                wvn = pw.tile([P, NKB, NF], BF16, tag="w")
                nc.sync.dma_start(
                    out=wvn,
                    in_=wv[:, n * NF:(n + 1) * NF].rearrange("(kb p) n -> p kb n", p=P),
                )
                for tb in range(C // P):
                    pg = pp2.tile([P, NF], F32, tag="pg")
                    for kb in range(NKB):
                        nc.tensor.matmul(
                            pg, xc[:, kb, tb * P:(tb + 1) * P], wvn[:, kb, :],
                            start=(kb == 0), stop=(kb == NKB - 1),
                        )
                    nc.vector.tensor_add(
                        vc[:, tb, n * NF:(n + 1) * NF], pg, bvt[:, n * NF:(n + 1) * NF]
                    )

        # ---- GEMM1 (q/k projections, feature-major) + attention, pipelined ----
        aoc = pao.tile([P, H, C], BF16, tag="aoc")
        if variant == "nog1":
            for tb in range(C // P):
                nc.vector.tensor_copy(
                    aoc[:, 4 * tb:4 * tb + 4, :],
                    vc[:, tb, :].rearrange("p (a b) -> p a b", a=4),
                )
        pending = []
        for h in range(H if variant != "nog1" else 0):
            qt = pqk.tile([P, C], BF16, tag="qk")
            kt = pqk.tile([P, C], BF16, tag="qk")
            for idx, dst in ((2 * h, qt), (2 * h + 1, kt)):
                wt = pwqk.tile([P, NKB, P], BF16, tag="wqk")
                nc.sync.dma_start(out=wt, in_=wqk[idx])
                pq = pp1.tile([P, C], F32, tag="pq")
                for kb in range(NKB):
                    nc.tensor.matmul(
                        pq, wt[:, kb, :], xc[:, kb, :],
                        start=(kb == 0), stop=(kb == NKB - 1),
                    )
                nc.vector.tensor_scalar(
                    out=dst, in0=pq, scalar1=bqkt[:, idx:idx + 1],
                    scalar2=None, op0=mybir.AluOpType.add,
                )
            pending.append((c, h, qt, kt, vc, aoc))
            if len(pending) > 1:
                attention(*pending.pop(0))
        if pending:
            attention(*pending.pop(0))

        # ---- GEMM3: y chunk [tokens, d] token-major ----
        for n in range(NN):
            won = pw.tile([P, NKB, NF], BF16, tag="w")
            nc.sync.dma_start(
                out=won,
                in_=wo[:, n * NF:(n + 1) * NF].rearrange("(kb p) n -> p kb n", p=P),
            )
            for tb in range(C // P):
                pg = pp2.tile([P, NF], F32, tag="pg")
                for kb in range(NKB):
                    nc.tensor.matmul(
                        pg, aoc[:, kb, tb * P:(tb + 1) * P], won[:, kb, :],
                        start=(kb == 0), stop=(kb == NKB - 1),
                    )
                yt = py.tile([P, NF], F32, tag="y")
                nc.vector.tensor_add(yt, pg, bot[:, n * NF:(n + 1) * NF])
                nc.sync.dma_start(
                    out=out[c * C + tb * P: c * C + (tb + 1) * P,
                            n * NF:(n + 1) * NF],
                    in_=yt,
                )


def build_nc(variant: str = "full"):
    nc = bacc.Bacc(
        "TRN2", target_bir_lowering=False, debug=False,
        enable_asserts=False, num_devices=NCORES,
    )
    aps = {
        "xT": nc.dram_tensor("xT", [D, T], BF16, kind="ExternalInput").ap(),
        "wqk": nc.dram_tensor("wqk", [2 * H, P, NKB, P], BF16, kind="ExternalInput").ap(),
        "wv": nc.dram_tensor("wv", [D, D], BF16, kind="ExternalInput").ap(),
        "wo": nc.dram_tensor("wo", [D, D], BF16, kind="ExternalInput").ap(),
        "bqk": nc.dram_tensor("bqk", [P, 2 * H], F32, kind="ExternalInput").ap(),
        "bv": nc.dram_tensor("bv", [D], BF16, kind="ExternalInput").ap(),
        "bo": nc.dram_tensor("bo", [D], F32, kind="ExternalInput").ap(),
        "mask": nc.dram_tensor("mask", [P, S], F32, kind="ExternalInput").ap(),
        "out": nc.dram_tensor("out", [T, D], F32, kind="ExternalOutput").ap(),
    }
    with tile.TileContext(nc) as tc:
        with ExitStack() as ctx:
            _emit(ctx, tc, aps, variant)
    nc.compile()
    return nc


def host_prep(x, W_qkv, b_qkv, W_out, b_out):
    """Build the per-core input maps (all host-side, numpy only)."""
    W_qkv = np.asarray(W_qkv, dtype=np.float32)
    W_out = np.asarray(W_out, dtype=np.float32)
    b_qkv = np.asarray(b_qkv, dtype=np.float32)
    b_out = np.asarray(b_out, dtype=np.float32)
    x = np.asarray(x, dtype=np.float32)

    W_q, W_k, W_v = W_qkv[:D], W_qkv[D:2 * D], W_qkv[2 * D:]
    # m-block order: q_0, k_0, q_1, k_1, ... (rows of W in head blocks)
    qk_rows = np.empty((2 * H, P, D), dtype=np.float32)
    qk_rows[0::2] = W_q.reshape(H, P, D)
    qk_rows[1::2] = W_k.reshape(H, P, D)
    # lhsT tiles: wqk[m, k_lo, kb, mcol] = W^T[kb*P+k_lo, m*P+mcol]
    #           = qk_rows[m, mcol, kb*P+k_lo]
    wqk = np.ascontiguousarray(
        qk_rows.reshape(2 * H, P, NKB, P).transpose(0, 3, 2, 1)
    ).astype(bf16)
    wv = np.ascontiguousarray(W_v.T).astype(bf16)
    wo = np.ascontiguousarray(W_out.T).astype(bf16)

    bqk = np.empty((2 * H, P), dtype=np.float32)
    bqk[0::2] = b_qkv[:D].reshape(H, P)
    bqk[1::2] = b_qkv[D:2 * D].reshape(H, P)
    bqk = np.ascontiguousarray(bqk.T)          # [P, 2H]
    bv = b_qkv[2 * D:].astype(bf16)
    bo = b_out
    mask = np.tile(np.triu(np.ones((S, S), dtype=np.float32)), (2, 1))

    in_maps = []
    for i in range(NCORES):
        xi = x[i * BL:(i + 1) * BL].reshape(T, D)
        xT = np.ascontiguousarray(xi.T).astype(bf16)
        in_maps.append({
            "xT": xT, "wqk": wqk, "wv": wv, "wo": wo,
            "bqk": bqk, "bv": bv, "bo": bo, "mask": mask,
        })
    return in_maps


def kernel(x, W_qkv, b_qkv, W_out, b_out):
    global LAST_RESULTS
    if "nc" not in _CACHE:
        _CACHE["nc"] = build_nc()
    nc = _CACHE["nc"]
    in_maps = host_prep(x, W_qkv, b_qkv, W_out, b_out)
    res = run_bass_kernel_spmd(nc, in_maps, core_ids=list(range(NCORES)))
    LAST_RESULTS = res
    outs = [res.results[i]["out"].reshape(BL, S, D) for i in range(NCORES)]
    return np.concatenate(outs, axis=0).astype(np.float32)


# revision 23
# speedup vs baseline: 1.2158x; 1.2158x over previous
"""Trainium2 Bass kernel for a causal attention head (softmax over the query
axis, scores scaled by sqrt(d_model)) with QKV/out projections.

Strategy: pure data-parallel over batch across 8 NeuronCores (32 batch rows
per core). All matmuls run in bf16 on the TensorEngine. Host-side
pre-processing (free, outside HW timing) transposes x and the weights into
feature-major layouts so that the kernel needs no on-chip transposes:

  per core:  xT [d, T]     (T = 32*64 = 2048 local tokens)
    GEMM1:   qkT = W_qk^T.T @ xT         -> [2d, T]  feature-major (per head)
    GEMM2:   vTok = xT.T @ W_v^T         -> [T, d]   token-major
    attn:    scoresT[k,q] = K^T.T @ Q^T  -> softmax over q = free axis
             outT[dv,q] = V.T @ A^T      -> attnOutT [d, T] feature-major
    GEMM3:   y = attnOutT.T @ W_out^T    -> [T, d]   token-major, DMA'd out

Everything is hardcoded for x:[256,64,2048], 16 heads, head_dim 128.
"""

import math
import os
from contextlib import ExitStack

import numpy as np
import ml_dtypes

import concourse.bass as bass
import concourse.bacc as bacc
import concourse.tile as tile
from concourse import mybir
from concourse.bass_utils import run_bass_kernel_spmd

BF16 = mybir.dt.bfloat16
F32 = mybir.dt.float32
bf16 = ml_dtypes.bfloat16

P = 128          # partitions / head_dim
D = 2048         # d_model
H = 16           # heads
S = 64           # sequence length
NCORES = 8
B = 256
BL = B // NCORES          # 32 batch rows per core
T = BL * S                # 2048 tokens per core
C = 512                   # chunk tokens
NCH = T // C              # 4 chunks per core
G = C // S                # 8 batch rows per chunk
NKB = D // P              # 16 contraction blocks
NF = 512                  # matmul moving free dim
NN = D // NF              # 4 n-blocks
SCALE = 1.0 / math.sqrt(D)

_CACHE: dict = {}
LAST_RESULTS = None


def _emit(ctx: ExitStack, tc: tile.TileContext, aps: dict, variant: str = "full"):
    nc = tc.nc
    xT, wqk, wv, wo, bqk, bv, bo, mask, out = (
        aps["xT"], aps["wqk"], aps["wv"], aps["wo"], aps["bqk"],
        aps["bv"], aps["bo"], aps["mask"], aps["out"],
    )

    px = ctx.enter_context(tc.tile_pool(name="px", bufs=2))
    pwqk = ctx.enter_context(tc.tile_pool(name="pwqk", bufs=3))
    pqk = ctx.enter_context(tc.tile_pool(name="pqk", bufs=6))
    pv_pool = ctx.enter_context(tc.tile_pool(name="pv", bufs=1))
    pw = ctx.enter_context(tc.tile_pool(name="pw", bufs=2))
    pao = ctx.enter_context(tc.tile_pool(name="pao", bufs=2))
    psm = ctx.enter_context(tc.tile_pool(name="psm", bufs=3))
    py = ctx.enter_context(tc.tile_pool(name="py", bufs=4))
    pc = ctx.enter_context(tc.tile_pool(name="pc", bufs=1))
    pp1 = ctx.enter_context(tc.tile_pool(name="pp1", bufs=2, space="PSUM"))
    pp2 = ctx.enter_context(tc.tile_pool(name="pp2", bufs=2, space="PSUM"))
    pps = ctx.enter_context(tc.tile_pool(name="pps", bufs=2, space="PSUM"))
    ppo = ctx.enter_context(tc.tile_pool(name="ppo", bufs=2, space="PSUM"))

    # ---- constants ----
    maskt = pc.tile([P, S], F32)
    nc.sync.dma_start(out=maskt, in_=mask)
    bqkt = pc.tile([P, 2 * H], F32)
    nc.sync.dma_start(out=bqkt, in_=bqk)
    bvt = pc.tile([P, D], BF16)
    nc.gpsimd.dma_start(
        out=bvt,
        in_=bass.AP(tensor=bv.tensor, offset=bv.offset, ap=[[0, P], bv.ap[0]]),
    )
    bot = pc.tile([P, D], F32)
    nc.gpsimd.dma_start(
        out=bot,
        in_=bass.AP(tensor=bo.tensor, offset=bo.offset, ap=[[0, P], bo.ap[0]]),
    )

    def attention_base0(c, h, qt, kt, vc, aoc):
        # all-base-0: scores [64, G, S]; V rows at base 0 (vc128 top / vcodd)
        vc128, vcodd = vc
        sp = pps.tile([S, G, S], F32, tag="sp")
        for g in range(G):
            nc.tensor.matmul(
                sp[:, g, :], kt[:, g * S:(g + 1) * S], qt[:, g * S:(g + 1) * S],
                start=True, stop=True,
            )
        e = psm.tile([S, G, S], F32, tag="e")
        nc.scalar.activation(e, sp, mybir.ActivationFunctionType.Exp, scale=SCALE)
        ez = psm.tile([S, G, S], F32, tag="ez")
        mask_b = bass.AP(
            tensor=maskt.tensor, offset=maskt.offset,
            ap=[[maskt.ap[0][0], S], [0, G], maskt.ap[1]],
        )
        nc.vector.tensor_mul(ez, e, mask_b)
        z = psm.tile([S, G], F32, tag="z")
        nc.vector.reduce_sum(z, ez, axis=mybir.AxisListType.X)
        r = psm.tile([S, G], F32, tag="r")
        nc.vector.reciprocal(r, z)
        a = psm.tile([S, G, S], BF16, tag="a")
        r_b = bass.AP(tensor=r.tensor, offset=r.offset,
                      ap=[r.ap[0], r.ap[1], [0, S]])
        nc.vector.tensor_mul(a, ez, r_b)
        op = ppo.tile([P, G, S], F32, tag="op")
        for g in range(G):
            vsl = (vc128[0:S, g // 2, h * P:(h + 1) * P] if g % 2 == 0
                   else vcodd[:, g // 2, h * P:(h + 1) * P])
            nc.tensor.matmul(
                op[:, g, :],
                vsl,
                a[:, g, :],
                start=True, stop=True,
            )
        nc.vector.tensor_copy(aoc[:, h, :], op)

    def attention(c, h, qt, kt, vc, aoc):
        if variant == "noattn":
            nc.vector.tensor_copy(aoc[:, h, :], qt)
            return
        if variant in ("base0", "full"):
            attention_base0(c, h, qt, kt, vc, aoc)
            return
        # scores^T for 8 batch rows: partitions = (half, k_tok), free = (j, q_tok)
        sp = pps.tile([P, G // 2, S], F32, tag="sp")
        for g in range(G):
            half, j = g % 2, g // 2
            nc.tensor.matmul(
                sp[half * S:(half + 1) * S, j, :],
                kt[:, g * S:(g + 1) * S],
                qt[:, g * S:(g + 1) * S],
                start=True, stop=True,
            )
        e = psm.tile([P, G // 2, S], F32, tag="e")
        nc.scalar.activation(e, sp, mybir.ActivationFunctionType.Exp, scale=SCALE)
        ez = psm.tile([P, G // 2, S], F32, tag="ez")
        mask_b = bass.AP(
            tensor=maskt.tensor, offset=maskt.offset,
            ap=[maskt.ap[0], [0, G // 2], maskt.ap[1]],
        )
        nc.vector.tensor_mul(ez, e, mask_b)
        z = psm.tile([P, G // 2], F32, tag="z")
        nc.vector.reduce_sum(z, ez, axis=mybir.AxisListType.X)
        r = psm.tile([P, G // 2], F32, tag="r")
        nc.vector.reciprocal(r, z)
        a = psm.tile([P, G // 2, S], BF16, tag="a")
        r_b = bass.AP(tensor=r.tensor, offset=r.offset,
                      ap=[r.ap[0], r.ap[1], [0, S]])
        nc.vector.tensor_mul(a, ez, r_b)

        if variant == "nopv":
            nc.vector.tensor_copy(aoc[:, h, 0:G // 2 * S], a)
            nc.vector.tensor_copy(aoc[:, h, G // 2 * S:], a)
            return

        op = ppo.tile([P, G, S], F32, tag="op")
        for g in range(G):
            half, j = g % 2, g // 2
            nc.tensor.matmul(
                op[:, g, :],
                vc[half * S:(half + 1) * S, g // 2, h * P:(h + 1) * P],
                a[half * S:(half + 1) * S, j, :],
                start=True, stop=True,
            )
        nc.vector.tensor_copy(aoc[:, h, :], op)

    for c in range(NCH):
        # ---- load x chunk (feature-major) ----
        xc = px.tile([P, NKB, C], BF16, tag="xc")
        for kb in range(NKB):
            nc.sync.dma_start(
                out=xc[:, kb, :],
                in_=xT[kb * P:(kb + 1) * P, c * C:(c + 1) * C],
            )

        # ---- GEMM2: vTok chunk, token-major ----
        if variant in ("base0", "full"):
            # packed M=128 GEMM2, then SBUF->SBUF DMA odd batch rows to base 0
            vc128 = pv_pool.tile([P, C // P, D], BF16, tag="vc")
            for n in range(NN):
                wvn = pw.tile([P, NKB, NF], BF16, tag="w")
                nc.sync.dma_start(
                    out=wvn,
                    in_=wv[:, n * NF:(n + 1) * NF].rearrange("(kb p) n -> p kb n", p=P),
                )
                for tb in range(C // P):
                    pg = pp2.tile([P, NF], F32, tag="pg")
                    for kb in range(NKB):
                        nc.tensor.matmul(
                            pg, xc[:, kb, tb * P:(tb + 1) * P], wvn[:, kb, :],
                            start=(kb == 0), stop=(kb == NKB - 1),
                        )
                    nc.vector.tensor_add(
                        vc128[:, tb, n * NF:(n + 1) * NF], pg,
                        bvt[:, n * NF:(n + 1) * NF],
                    )
            vcodd = pv_pool.tile([S, C // P, D], BF16, tag="vcodd")
            for tb in range(C // P):
                nc.sync.dma_start(out=vcodd[:, tb, :], in_=vc128[S:P, tb, :])
            vc = (vc128, vcodd)
        else:
            vc = pv_pool.tile([P, C // P, D], BF16, tag="vc")
            for n in range(NN):
                wvn = pw.tile([P, NKB, NF], BF16, tag="w")
                nc.sync.dma_start(
                    out=wvn,
                    in_=wv[:, n * NF:(n + 1) * NF].rearrange("(kb p) n -> p kb n", p=P),
                )
                for tb in range(C // P):
                    pg = pp2.tile([P, NF], F32, tag="pg")
                    for kb in range(NKB):
                        nc.tensor.matmul(
                            pg, xc[:, kb, tb * P:(tb + 1) * P], wvn[:, kb, :],
                            start=(kb == 0), stop=(kb == NKB - 1),
                        )
                    nc.vector.tensor_add(
                        vc[:, tb, n * NF:(n + 1) * NF], pg, bvt[:, n * NF:(n + 1) * NF]
                    )

        # ---- GEMM1 (q/k projections, feature-major) + attention, pipelined ----
        aoc = pao.tile([P, H, C], BF16, tag="aoc")
        if variant == "nog1":
            for tb in range(C // P):
                nc.vector.tensor_copy(
                    aoc[:, 4 * tb:4 * tb + 4, :],
                    vc[:, tb, :].rearrange("p (a b) -> p a b", a=4),
                )
        pending = []
        for h in range(H if variant != "nog1" else 0):
            qt = pqk.tile([P, C], BF16, tag="qk")
            kt = pqk.tile([P, C], BF16, tag="qk")
            for idx, dst in ((2 * h, qt), (2 * h + 1, kt)):
                wt = pwqk.tile([P, NKB, P], BF16, tag="wqk")
                nc.sync.dma_start(out=wt, in_=wqk[idx])
                pq = pp1.tile([P, C], F32, tag="pq")
                for kb in range(NKB):
                    nc.tensor.matmul(
                        pq, wt[:, kb, :], xc[:, kb, :],
                        start=(kb == 0), stop=(kb == NKB - 1),
                    )
                nc.vector.tensor_scalar(
                    out=dst, in0=pq, scalar1=bqkt[:, idx:idx + 1],
                    scalar2=None, op0=mybir.AluOpType.add,
                )
            pending.append((c, h, qt, kt, vc, aoc))
            if len(pending) > 1:
                attention(*pending.pop(0))
        if pending:
            attention(*pending.pop(0))

        # ---- GEMM3: y chunk [tokens, d] token-major ----
        for n in range(NN):
            won = pw.tile([P, NKB, NF], BF16, tag="w")
            nc.sync.dma_start(
                out=won,
                in_=wo[:, n * NF:(n + 1) * NF].rearrange("(kb p) n -> p kb n", p=P),
            )
            for tb in range(C // P):
                pg = pp2.tile([P, NF], F32, tag="pg")
                for kb in range(NKB):
                    nc.tensor.matmul(
                        pg, aoc[:, kb, tb * P:(tb + 1) * P], won[:, kb, :],
                        start=(kb == 0), stop=(kb == NKB - 1),
                    )
                yt = py.tile([P, NF], F32, tag="y")
                nc.vector.tensor_add(yt, pg, bot[:, n * NF:(n + 1) * NF])
                nc.sync.dma_start(
                    out=out[c * C + tb * P: c * C + (tb + 1) * P,
                            n * NF:(n + 1) * NF],
                    in_=yt,
                )


def build_nc(variant: str = "full"):
    nc = bacc.Bacc(
        "TRN2", target_bir_lowering=False, debug=False,
        enable_asserts=False, num_devices=NCORES,
    )
    aps = {
        "xT": nc.dram_tensor("xT", [D, T], BF16, kind="ExternalInput").ap(),
        "wqk": nc.dram_tensor("wqk", [2 * H, P, NKB, P], BF16, kind="ExternalInput").ap(),
        "wv": nc.dram_tensor("wv", [D, D], BF16, kind="ExternalInput").ap(),
        "wo": nc.dram_tensor("wo", [D, D], BF16, kind="ExternalInput").ap(),
        "bqk": nc.dram_tensor("bqk", [P, 2 * H], F32, kind="ExternalInput").ap(),
        "bv": nc.dram_tensor("bv", [D], BF16, kind="ExternalInput").ap(),
        "bo": nc.dram_tensor("bo", [D], F32, kind="ExternalInput").ap(),
        "mask": nc.dram_tensor("mask", [P, S], F32, kind="ExternalInput").ap(),
        "out": nc.dram_tensor("out", [T, D], F32, kind="ExternalOutput").ap(),
    }
    with tile.TileContext(nc) as tc:
        with ExitStack() as ctx:
            _emit(ctx, tc, aps, variant)
    nc.compile()
    return nc


def host_prep(x, W_qkv, b_qkv, W_out, b_out):
    """Build the per-core input maps (all host-side, numpy only)."""
    W_qkv = np.asarray(W_qkv, dtype=np.float32)
    W_out = np.asarray(W_out, dtype=np.float32)
    b_qkv = np.asarray(b_qkv, dtype=np.float32)
    b_out = np.asarray(b_out, dtype=np.float32)
    x = np.asarray(x, dtype=np.float32)

    W_q, W_k, W_v = W_qkv[:D], W_qkv[D:2 * D], W_qkv[2 * D:]
    # m-block order: q_0, k_0, q_1, k_1, ... (rows of W in head blocks)
    qk_rows = np.empty((2 * H, P, D), dtype=np.float32)
    qk_rows[0::2] = W_q.reshape(H, P, D)
    qk_rows[1::2] = W_k.reshape(H, P, D)
    # lhsT tiles: wqk[m, k_lo, kb, mcol] = W^T[kb*P+k_lo, m*P+mcol]
    #           = qk_rows[m, mcol, kb*P+k_lo]
    wqk = np.ascontiguousarray(
        qk_rows.reshape(2 * H, P, NKB, P).transpose(0, 3, 2, 1)
    ).astype(bf16)
    wv = np.ascontiguousarray(W_v.T).astype(bf16)
    wo = np.ascontiguousarray(W_out.T).astype(bf16)

    bqk = np.empty((2 * H, P), dtype=np.float32)
    bqk[0::2] = b_qkv[:D].reshape(H, P)
    bqk[1::2] = b_qkv[D:2 * D].reshape(H, P)
    bqk = np.ascontiguousarray(bqk.T)          # [P, 2H]
    bv = b_qkv[2 * D:].astype(bf16)
    bo = b_out
    mask = np.tile(np.triu(np.ones((S, S), dtype=np.float32)), (2, 1))

    in_maps = []
    for i in range(NCORES):
        xi = x[i * BL:(i + 1) * BL].reshape(T, D)
        xT = np.ascontiguousarray(xi.T).astype(bf16)
        in_maps.append({
            "xT": xT, "wqk": wqk, "wv": wv, "wo": wo,
            "bqk": bqk, "bv": bv, "bo": bo, "mask": mask,
        })
    return in_maps


def kernel(x, W_qkv, b_qkv, W_out, b_out):
    global LAST_RESULTS
    if "nc" not in _CACHE:
        _CACHE["nc"] = build_nc()
    nc = _CACHE["nc"]
    in_maps = host_prep(x, W_qkv, b_qkv, W_out, b_out)
    res = run_bass_kernel_spmd(nc, in_maps, core_ids=list(range(NCORES)))
    LAST_RESULTS = res
    outs = [res.results[i]["out"].reshape(BL, S, D) for i in range(NCORES)]
    return np.concatenate(outs, axis=0).astype(np.float32)
